# revision 11
# baseline (speedup 1.0000x reference)
"""Trainium2 Bass kernel for nn_Cross_Modal_Attention_Fusion.

Strategy (2 launches over 8 NeuronCores):
  Launch 1 (data-parallel, 2 batches/core): fc1, 1x1 convs, gated
    cross-modal scores, exp (with the global gate-normalization scalar
    folded into the activation scale), partial exp-sums, the value bmm
    Z^T = o @ V^T (global attention normalization deferred as a scalar),
    and the fp32 softpool pieces (exp / x*exp, spatially pooled) of the
    raw inputs. Attention-path matmuls run in bf16: the global exp-sum
    normalizations scale the attention branch to ~1e-13, far below the
    fp32 resolution of the `+ x_sp` residuals, so bf16 precision there
    cannot perturb the final output; the live path (softpool of raw
    x/y, fc2, softmax) stays fp32.
  Host: sums the per-core exp partial sums (the "all-reduce"), reshards
    batch windows (softpool batch window 4, stride 2).
  Launch 2 (one output plane/core on 7 cores): batch-window combine of
    pooled pieces, softpool division, residual adds, fc2 (fp32),
    softmax, feature scaling.
"""

import contextlib
import ctypes
import sys
import types

import numpy as np
import ml_dtypes

import concourse.bass as bass
import concourse.bass_isa as bass_isa
import concourse.mybir as mybir
import concourse.tile as tile
from concourse.bass_utils import run_bass_kernel_spmd
from concourse.masks import make_identity
from concourse.vector_clock import ScopedClock, VectorClock

F32 = mybir.dt.float32
BF16 = mybir.dt.bfloat16
AF = mybir.ActivationFunctionType
ALU = mybir.AluOpType
AX = mybir.AxisListType
BF = ml_dtypes.bfloat16

P = 128
B, L, D, H, M = 16, 512, 1024, 1024, 512
NCORES = 8
BPC = 2  # batches per core, launch 1
PLANES = 7  # softpool output batch planes

# ---------------------------------------------------------------------------
# toolchain workarounds -----------------------------------------------------
# ---------------------------------------------------------------------------


def _split_drain_and_barrier(self, tick_clock, wait_clock):
    # walrus here caps sync commands per instruction; split the exit
    # drain's global-clock waits across single-wait nops.
    nc = self.nc
    gc = tick_clock.global_clock
    n = len(gc)
    for p in range(n):
        t = gc[p]
        if t > 0:
            vc = VectorClock([0] * n)
            vc.require_at_least(p, t)
            nop = nc.sync.nop(nofuse=True, hint="drain_split")
            wait_clock.add_sem_waits(nop.ins, ScopedClock({None: vc}))
    nc.sync.drain()
    nc.all_engine_barrier()
    assert self.sems is not None
    popped = nc._tile_sem_poison_stack.pop()
    assert popped is self._sem_poison
    nc.clear_and_free_semaphores(list(self.sems.allocated().values()))
    nc.all_engine_barrier()


tile.TileContext._drain_and_barrier = _split_drain_and_barrier

_wsplit_ctr = [0]


def _split_sync_waits(nc, max_waits=1):
    # Hoist extra per-instruction waits onto preceding same-engine nops.
    for f in nc.m.functions:
        for bb in f.blocks:
            changed = False
            newlist = []
            for ins in bb.instructions:
                si = ins.sync_info
                if si is not None:
                    waits = list(si.on_wait)
                    ups = list(si.on_update)
                    assert len(ups) <= 1, f"{ins.name}: {len(ups)} updates"
                    if len(waits) > max_waits:
                        for w in waits[:-max_waits]:
                            _wsplit_ctr[0] += 1
                            nop = mybir.InstNoOp(
                                name=f"I-wsplit-{_wsplit_ctr[0]}", engine=ins.engine
                            )
                            nop.sync_info = mybir.SyncInfo(on_wait=[w], on_update=[])
                            newlist.append(nop)
                        ins.sync_info = mybir.SyncInfo(
                            on_wait=waits[-max_waits:], on_update=ups
                        )
                        changed = True
                newlist.append(ins)
            if changed:
                bb.instructions = newlist
    return nc


def _install_profile_hook():
    # Provide the missing antenv.axon_hooks module so trace=True works.
    if "antenv.axon_hooks" in sys.modules:
        return
    so_path = "/opt/axon/libaxon_pjrt.so"
    try:
        lib = ctypes.CDLL(so_path)
    except OSError:
        return
    if not hasattr(lib, "axon_start_nrt_profile"):
        return
    lib.axon_start_nrt_profile.argtypes = [
        ctypes.POINTER(ctypes.c_int64),
        ctypes.c_size_t,
    ]
    lib.axon_start_nrt_profile.restype = ctypes.c_int64
    lib.axon_stop_nrt_profile.argtypes = [ctypes.c_char_p]
    lib.axon_stop_nrt_profile.restype = ctypes.c_int64

    @contextlib.contextmanager
    def _hook(output_dir, device_ids):
        import jax

        jax.devices()
        if device_ids:
            ids = (ctypes.c_int64 * len(device_ids))(*device_ids)
            rc = lib.axon_start_nrt_profile(ids, len(device_ids))
        else:
            rc = lib.axon_start_nrt_profile(None, 0)
        if rc != 0:
            raise RuntimeError(f"axon_start_nrt_profile rc={rc}")
        try:
            yield
        finally:
            n = lib.axon_stop_nrt_profile(str(output_dir).encode())
            if n < 0:
                raise RuntimeError(f"axon_stop_nrt_profile rc={n}")

    mod = types.ModuleType("antenv.axon_hooks")
    mod.get_axon_ntff_profile_hook = lambda: _hook
    mod.set_axon_ntff_profile_hook = lambda h: None
    sys.modules["antenv.axon_hooks"] = mod
    import antenv

    antenv.axon_hooks = mod


_install_profile_hook()


# ---------------------------------------------------------------------------
# launch 1 ------------------------------------------------------------------
# ---------------------------------------------------------------------------


def _sb(dram_ap, p=P):
    """View a [R, C] dram AP as [p, R//p, C] (row = s*p + q)."""
    return dram_ap.rearrange("(s p) c -> p s c", p=p)


def gen_launch1():
    nc = bass.Bass("TRN2", target_bir_lowering=False, debug=False, num_devices=NCORES)

    xT = nc.dram_tensor("xT", [BPC, D, L], BF16, kind="ExternalInput")
    yT = nc.dram_tensor("yT", [BPC, D, L], BF16, kind="ExternalInput")
    xpT = nc.dram_tensor("xpT", [BPC, D // 2, L * 2], F32, kind="ExternalInput")
    ypT = nc.dram_tensor("ypT", [BPC, D // 2, L * 2], F32, kind="ExternalInput")
    w1tT = nc.dram_tensor("w1tT", [D, H], BF16, kind="ExternalInput")
    w1vT = nc.dram_tensor("w1vT", [D, H], BF16, kind="ExternalInput")
    cw = {
        n: nc.dram_tensor(n, [L, L], BF16, kind="ExternalInput")
        for n in ("wqtT", "wktT", "wvtT", "wqvT", "wkvT", "wvvT")
    }
    gates = {
        n: nc.dram_tensor(n, [L, H], BF16, kind="ExternalInput")
        for n in ("gt0", "gt1", "gv0", "gv1")
    }
    gT = {
        n: nc.dram_tensor(n, [H, L], BF16, kind="ExternalInput")
        for n in ("gt2T", "gv2T")
    }
    b1t = nc.dram_tensor("b1t", [1, H], F32, kind="ExternalInput")
    b1v = nc.dram_tensor("b1v", [1, H], F32, kind="ExternalInput")
    cscal = nc.dram_tensor("cscal", [P, 1], F32, kind="ExternalInput")

    ZxT = nc.dram_tensor("ZxT", [BPC, H // 2, L * 2], BF16, kind="ExternalOutput")
    ZyT = nc.dram_tensor("ZyT", [BPC, H // 2, L * 2], BF16, kind="ExternalOutput")
    pool_out = {
        n: nc.dram_tensor(n, [BPC, D // 2, L // 2], F32, kind="ExternalOutput")
        for n in ("pxn", "pxd", "pyn", "pyd")
    }
    sums = nc.dram_tensor("sums", [1, 2], F32, kind="ExternalOutput")

    with tile.TileContext(nc) as tc:
        with contextlib.ExitStack() as ctx:
            weights = ctx.enter_context(tc.tile_pool(name="weights", bufs=1))
            psum = ctx.enter_context(tc.tile_pool(name="psum", bufs=7, space="PSUM"))
            psum1 = ctx.enter_context(tc.tile_pool(name="psum1", bufs=1, space="PSUM"))
            pin = ctx.enter_context(tc.tile_pool(name="pin", bufs=2))
            pexp = ctx.enter_context(tc.tile_pool(name="pexp", bufs=2))
            pterm = ctx.enter_context(tc.tile_pool(name="pterm", bufs=2))
            ppoolout = ctx.enter_context(tc.tile_pool(name="ppoolout", bufs=4))
            pXT = ctx.enter_context(tc.tile_pool(name="pXT", bufs=3))
            pxh = ctx.enter_context(tc.tile_pool(name="pxh", bufs=2))
            pqk = ctx.enter_context(tc.tile_pool(name="pqk", bufs=4))
            pvT = ctx.enter_context(tc.tile_pool(name="pvT", bufs=2))
            po = ctx.enter_context(tc.tile_pool(name="po", bufs=2))
            pg = ctx.enter_context(tc.tile_pool(name="pg", bufs=4))
            pz = ctx.enter_context(tc.tile_pool(name="pz", bufs=4))
            psc = ctx.enter_context(tc.tile_pool(name="psc", bufs=1))

            W1t = weights.tile([P, 8, H], BF16, tag="w1")
            nc.sync.dma_start(out=W1t[:], in_=_sb(w1tT.ap()))
            W1v = weights.tile([P, 8, H], BF16, tag="w1v")
            nc.sync.dma_start(out=W1v[:], in_=_sb(w1vT.ap()))
            CW = {}
            for n, t in cw.items():
                CW[n] = weights.tile([P, 4, L], BF16, tag=f"cw_{n}", name=f"cw_{n}")
                nc.sync.dma_start(out=CW[n][:], in_=_sb(t.ap()))
            B1t = weights.tile([P, H], F32, tag="b1t")
            nc.sync.dma_start(out=B1t[:], in_=b1t.ap().to_broadcast((P, H)))
            B1v = weights.tile([P, H], F32, tag="b1v")
            nc.sync.dma_start(out=B1v[:], in_=b1v.ap().to_broadcast((P, H)))
            Cs = weights.tile([P, 1], F32, tag="cs")
            nc.sync.dma_start(out=Cs[:], in_=cscal.ap())
            sxa = psc.tile([P, 2, 2, 16], F32, tag="sxa")  # [b, modal, col]

            for b in range(BPC):
                # ---- raw-input softpool pieces (fp32, live path) ----
                for pT, outn, outd in ((xpT, "pxn", "pxd"), (ypT, "pyn", "pyd")):
                    for pt in range(4):
                        tin = pin.tile([P, L * 2], F32, tag="pin")
                        nc.sync.dma_start(
                            out=tin[:], in_=pT.ap()[b, pt * P : (pt + 1) * P, :]
                        )
                        te = pexp.tile([P, L * 2], F32, tag="pe")
                        nc.scalar.activation(te[:], tin[:], AF.Exp)
                        tt = pterm.tile([P, L * 2], F32, tag="pt")
                        nc.vector.tensor_mul(tt[:], te[:], tin[:])
                        # free dims (dp, l) = (dp, l2, lp); keep l2
                        tn = ppoolout.tile([P, L // 2], F32, tag="pon")
                        td = ppoolout.tile([P, L // 2], F32, tag="pod")
                        nc.vector.tensor_reduce(
                            tn[:],
                            tt.rearrange("p (dp l2 lp) -> p l2 dp lp", dp=2, lp=2),
                            axis=AX.XY,
                            op=ALU.add,
                        )
                        nc.vector.tensor_reduce(
                            td[:],
                            te.rearrange("p (dp l2 lp) -> p l2 dp lp", dp=2, lp=2),
                            axis=AX.XY,
                            op=ALU.add,
                        )
                        nc.sync.dma_start(
                            out=_sb(pool_out[outn].ap()[b])[:, pt, :], in_=tn[:]
                        )
                        nc.sync.dma_start(
                            out=_sb(pool_out[outd].ap()[b])[:, pt, :], in_=td[:]
                        )

                # ---- fc1 ----
                def fc1(src_dram, Wt, Bt, dst):
                    for mi in range(4):
                        kt = pXT.tile([P, 8, P], BF16, tag="xt")
                        nc.sync.dma_start(
                            out=kt[:],
                            in_=_sb(src_dram.ap()[b])[:, :, mi * P : (mi + 1) * P],
                        )
                        for ni in range(2):
                            ps = psum.tile([P, 512], F32, tag="ps")
                            for ks in range(8):
                                nc.tensor.matmul(
                                    ps[:],
                                    lhsT=kt[:, ks, :],
                                    rhs=Wt[:, ks, ni * 512 : (ni + 1) * 512],
                                    start=(ks == 0),
                                    stop=(ks == 7),
                                )
                            nc.vector.tensor_add(
                                dst[:, mi, ni * 512 : (ni + 1) * 512],
                                ps[:],
                                Bt[:, ni * 512 : (ni + 1) * 512],
                            )

                xh = pxh.tile([P, 4, H], BF16, tag="xh")
                fc1(xT, W1t, B1t, xh)
                yh = pxh.tile([P, 4, H], BF16, tag="xh")
                fc1(yT, W1v, B1v, yh)

                # ---- gated 1x1 convs ----
                def conv(wt, src, gate_dram, dst):
                    for mi in range(4):
                        for ni in range(2):
                            ps = psum.tile([P, 512], F32, tag="ps")
                            for ks in range(4):
                                nc.tensor.matmul(
                                    ps[:],
                                    lhsT=wt[:, ks, mi * P : (mi + 1) * P],
                                    rhs=src[:, ks, ni * 512 : (ni + 1) * 512],
                                    start=(ks == 0),
                                    stop=(ks == 3),
                                )
                            g = pg.tile([P, 512], BF16, tag="g")
                            nc.sync.dma_start(
                                out=g[:],
                                in_=gate_dram.ap()[
                                    mi * P : (mi + 1) * P,
                                    ni * 512 : (ni + 1) * 512,
                                ],
                            )
                            nc.vector.tensor_mul(
                                dst[:, mi, ni * 512 : (ni + 1) * 512], ps[:], g[:]
                            )

                q_t = pqk.tile([P, 4, H], BF16, tag="qk")
                conv(CW["wqtT"], xh, gates["gt0"], q_t)
                k_t = pqk.tile([P, 4, H], BF16, tag="qk")
                conv(CW["wktT"], xh, gates["gt1"], k_t)
                q_v = pqk.tile([P, 4, H], BF16, tag="qk")
                conv(CW["wqvT"], yh, gates["gv0"], q_v)
                k_v = pqk.tile([P, 4, H], BF16, tag="qk")
                conv(CW["wkvT"], yh, gates["gv1"], k_v)

                # ---- transposed gated value convs: v^T[h, l] ----
                def vconv(src, wt, gT_dram, dst):
                    for mi in range(8):
                        ps = psum.tile([P, 512], F32, tag="ps")
                        for ks in range(4):
                            nc.tensor.matmul(
                                ps[:],
                                lhsT=src[:, ks, mi * P : (mi + 1) * P],
                                rhs=wt[:, ks, :],
                                start=(ks == 0),
                                stop=(ks == 3),
                            )
                        g = pg.tile([P, 512], BF16, tag="g")
                        nc.sync.dma_start(
                            out=g[:], in_=gT_dram.ap()[mi * P : (mi + 1) * P, :]
                        )
                        nc.vector.tensor_mul(dst[:, mi, :], ps[:], g[:])

                vtT = pvT.tile([P, 8, L], BF16, tag="vT")
                vconv(xh, CW["wvtT"], gT["gt2T"], vtT)
                vvT = pvT.tile([P, 8, L], BF16, tag="vT")
                vconv(yh, CW["wvvT"], gT["gv2T"], vvT)

                # ---- scores -> exp -> Z^T, per k'-half ----
                def score_z(kk, qq, vT, zdram_b, acc_sel):
                    zv = zdram_b.rearrange("k2 (kp l) -> (k2 kp) l", kp=2)
                    for nh in range(2):
                        oh = po.tile([P, 8, 512], BF16, tag="o")
                        for mi in range(8):
                            ps = psum.tile([P, 512], F32, tag="ps")
                            for ks in range(4):
                                nc.tensor.matmul(
                                    ps[:],
                                    lhsT=kk[:, ks, mi * P : (mi + 1) * P],
                                    rhs=qq[:, ks, nh * 512 : (nh + 1) * 512],
                                    start=(ks == 0),
                                    stop=(ks == 3),
                                )
                            col = nh * 8 + mi
                            nc.scalar.activation(
                                oh[:, mi, :],
                                ps[:],
                                AF.Exp,
                                scale=Cs[:, 0:1],
                                accum_out=sxa[:, b, acc_sel, col : col + 1],
                            )
                        for lk in range(4):
                            ps = psum.tile([P, 512], F32, tag="ps")
                            for ks in range(8):
                                nc.tensor.matmul(
                                    ps[:],
                                    lhsT=oh[:, ks, lk * P : (lk + 1) * P],
                                    rhs=vT[:, ks, :],
                                    start=(ks == 0),
                                    stop=(ks == 7),
                                )
                            st = pz.tile([P, 512], BF16, tag="z")
                            nc.scalar.activation(st[:], ps[:], AF.Copy)
                            r0 = nh * 512 + lk * P
                            nc.sync.dma_start(
                                out=zv[r0 : r0 + P, :], in_=st[:]
                            )

                score_z(k_v, q_t, vvT, ZxT.ap()[b], 0)
                score_z(k_t, q_v, vtT, ZyT.ap()[b], 1)

            # ---- partial exp sums ----
            stot = psc.tile([P, 2], F32, tag="stot")
            nc.vector.tensor_reduce(
                stot[:, 0:1], sxa[:, :, 0, :], axis=AX.XY, op=ALU.add
            )
            nc.vector.tensor_reduce(
                stot[:, 1:2], sxa[:, :, 1, :], axis=AX.XY, op=ALU.add
            )
            ones = psc.tile([P, 1], F32, tag="ones")
            nc.vector.memset(ones[:], 1.0)
            pred = psum1.tile([P, 2], F32, tag="pred")
            nc.tensor.matmul(
                pred[0:1, :], lhsT=ones[:], rhs=stot[:], start=True, stop=True
            )
            sred = psc.tile([1, 2], F32, tag="sred")
            nc.vector.tensor_copy(sred[:], pred[0:1, :])
            nc.sync.dma_start(out=sums.ap()[:, :], in_=sred[:])

    _split_sync_waits(nc)
    return nc


# ---------------------------------------------------------------------------
# launch 2 ------------------------------------------------------------------
# ---------------------------------------------------------------------------


def gen_launch2():
    nc = bass.Bass("TRN2", target_bir_lowering=False, debug=False, num_devices=NCORES)

    ZxT4 = nc.dram_tensor("ZxT4", [4, H // 2, L * 2], BF16, kind="ExternalInput")
    ZyT4 = nc.dram_tensor("ZyT4", [4, H // 2, L * 2], BF16, kind="ExternalInput")
    pools = {
        n: nc.dram_tensor(n, [4, D // 2, L // 2], F32, kind="ExternalInput")
        for n in ("pxn", "pxd", "pyn", "pyd")
    }
    w2vT = nc.dram_tensor("w2vT", [H // 2, M], F32, kind="ExternalInput")
    w2tT = nc.dram_tensor("w2tT", [H // 2, M], F32, kind="ExternalInput")
    b2v = nc.dram_tensor("b2v", [1, M], F32, kind="ExternalInput")
    b2t = nc.dram_tensor("b2t", [1, M], F32, kind="ExternalInput")
    kscal = nc.dram_tensor("kscal", [P, 2], F32, kind="ExternalInput")
    wscal = nc.dram_tensor("wscal", [P, 2], F32, kind="ExternalInput")

    xhat = nc.dram_tensor("xhat", [L // 2, M], F32, kind="ExternalOutput")
    yhat = nc.dram_tensor("yhat", [L // 2, M], F32, kind="ExternalOutput")
    fxo = nc.dram_tensor("fxo", [L // 2, M], F32, kind="ExternalOutput")
    fyo = nc.dram_tensor("fyo", [L // 2, M], F32, kind="ExternalOutput")

    with tile.TileContext(nc) as tc:
        with contextlib.ExitStack() as ctx:
            weights = ctx.enter_context(tc.tile_pool(name="weights", bufs=1))
            psum = ctx.enter_context(tc.tile_pool(name="psum", bufs=3, space="PSUM"))
            pzt = ctx.enter_context(tc.tile_pool(name="pzt", bufs=2))
            pmt = ctx.enter_context(tc.tile_pool(name="pmt", bufs=2))
            pet = ctx.enter_context(tc.tile_pool(name="pet", bufs=2))
            ptt = ctx.enter_context(tc.tile_pool(name="ptt", bufs=2))
            pacc = ctx.enter_context(tc.tile_pool(name="pacc", bufs=2))
            psp = ctx.enter_context(tc.tile_pool(name="psp", bufs=2))
            pmisc = ctx.enter_context(tc.tile_pool(name="pmisc", bufs=2))

            W2v = weights.tile([P, 4, M], F32, tag="w2v")
            nc.sync.dma_start(out=W2v[:], in_=_sb(w2vT.ap()))
            W2t = weights.tile([P, 4, M], F32, tag="w2t")
            nc.sync.dma_start(out=W2t[:], in_=_sb(w2tT.ap()))
            B2v = weights.tile([P, M], F32, tag="b2v")
            nc.sync.dma_start(out=B2v[:], in_=b2v.ap().to_broadcast((P, M)))
            B2t = weights.tile([P, M], F32, tag="b2t")
            nc.sync.dma_start(out=B2t[:], in_=b2t.ap().to_broadcast((P, M)))
            Ks = weights.tile([P, 2], F32, tag="ks")
            nc.sync.dma_start(out=Ks[:], in_=kscal.ap())
            Ws = weights.tile([P, 2], F32, tag="ws")
            nc.sync.dma_start(out=Ws[:], in_=wscal.ap())
            ident = weights.tile([P, P], F32, tag="ident")
            make_identity(nc, ident)

            for modal, (Z4, pn, pd, W2, B2, hat_d, f_d) in enumerate(
                (
                    (ZxT4, "pxn", "pxd", W2v, B2v, xhat, fxo),
                    (ZyT4, "pyn", "pyd", W2t, B2t, yhat, fyo),
                )
            ):
                # ---- attention softpool pieces over the 4-batch window ----
                numA = pacc.tile([P, 4, L // 2], F32, tag="numA")
                denA = pacc.tile([P, 4, L // 2], F32, tag="denA")
                for pt in range(4):
                    zt = pzt.tile([P, 4, L * 2], BF16, tag="zt")
                    for j in range(4):
                        nc.sync.dma_start(
                            out=zt[:, j, :],
                            in_=Z4.ap()[j, pt * P : (pt + 1) * P, :],
                        )
                    mt = pmt.tile([P, 4, L * 2], BF16, tag="mt")
                    nc.vector.tensor_scalar_mul(mt[:], zt[:], Ks[:, modal : modal + 1])
                    et = pet.tile([P, 4, L * 2], BF16, tag="et")
                    nc.scalar.activation(et[:], mt[:], AF.Exp)
                    tt = ptt.tile([P, 4, L * 2], BF16, tag="tt")
                    nc.vector.tensor_mul(tt[:], et[:], mt[:])
                    # free (j, kp, l2, lp); keep l2
                    nc.vector.tensor_reduce(
                        numA[:, pt, :],
                        tt.rearrange(
                            "p j (kp l2 lp) -> p l2 j kp lp", kp=2, lp=2
                        ),
                        axis=AX.XYZ,
                        op=ALU.add,
                    )
                    nc.vector.tensor_reduce(
                        denA[:, pt, :],
                        et.rearrange(
                            "p j (kp l2 lp) -> p l2 j kp lp", kp=2, lp=2
                        ),
                        axis=AX.XYZ,
                        op=ALU.add,
                    )

                # ---- raw softpool: sum pieces over window, divide ----
                def sum4(name):
                    acc = psp.tile([P, 4, L // 2], F32, tag="sp4")
                    nc.sync.dma_start(out=acc[:], in_=_sb(pools[name].ap()[0]))
                    for j in range(1, 4):
                        tj = psp.tile([P, 4, L // 2], F32, tag="sp4j")
                        nc.sync.dma_start(out=tj[:], in_=_sb(pools[name].ap()[j]))
                        nc.vector.tensor_add(acc[:], acc[:], tj[:])
                    return acc

                nS = sum4(pn)
                dS = sum4(pd)
                rD = psp.tile([P, 4, L // 2], F32, tag="rd")
                nc.vector.reciprocal(rD[:], dS[:])
                xspT = psp.tile([P, 4, L // 2], F32, tag="xspT")
                nc.vector.tensor_mul(xspT[:], nS[:], rD[:])

                rA = psp.tile([P, 4, L // 2], F32, tag="ra")
                nc.vector.reciprocal(rA[:], denA[:])
                attn = psp.tile([P, 4, L // 2], F32, tag="attn")
                nc.vector.tensor_mul(attn[:], numA[:], rA[:])

                pre1 = psp.tile([P, 4, L // 2], F32, tag="pre1")
                nc.vector.tensor_add(pre1[:], attn[:], xspT[:])

                # ---- fc2 (fp32) ----
                hat = pmisc.tile([P, 2, M], F32, tag="hat")
                for mt_i in range(2):
                    ps = psum.tile([P, 512], F32, tag="ps")
                    for ks in range(4):
                        nc.tensor.matmul(
                            ps[:],
                            lhsT=pre1[:, ks, mt_i * P : (mt_i + 1) * P],
                            rhs=W2[:, ks, :],
                            start=(ks == 0),
                            stop=(ks == 3),
                        )
                    nc.vector.tensor_add(hat[:, mt_i, :], ps[:], B2[:, :])

                if modal == 0:
                    # transpose x_spT -> x_sp and add the second residual
                    xsp = pmisc.tile([P, 2, M], F32, tag="xsp")
                    for hb in range(4):
                        for lb in range(2):
                            pst = psum.tile([P, P], F32, tag="pst")
                            nc.tensor.transpose(
                                pst[:],
                                xspT[:, hb, lb * P : (lb + 1) * P],
                                ident[:],
                            )
                            nc.vector.tensor_copy(
                                xsp[:, lb, hb * P : (hb + 1) * P], pst[:]
                            )
                    nc.vector.tensor_add(hat[:], hat[:], xsp[:])

                # ---- softmax over m + feature scaling ----
                for mt_i in range(2):
                    mx = pmisc.tile([P, 1], F32, tag="mx")
                    nc.vector.tensor_reduce(
                        mx[:], hat[:, mt_i, :], axis=AX.X, op=ALU.max
                    )
                    nmx = pmisc.tile([P, 1], F32, tag="nmx")
                    nc.vector.tensor_scalar_mul(nmx[:], mx[:], -1.0)
                    eh = pmisc.tile([P, M], F32, tag="eh")
                    rs = pmisc.tile([P, 1], F32, tag="rs")
                    nc.scalar.activation(
                        eh[:], hat[:, mt_i, :], AF.Exp, bias=nmx[:, 0:1],
                        accum_out=rs[:, 0:1],
                    )
                    ri = pmisc.tile([P, 1], F32, tag="ri")
                    nc.vector.reciprocal(ri[:], rs[:])
                    sm = pmisc.tile([P, M], F32, tag="sm")
                    nc.vector.tensor_mul(sm[:], eh[:], ri[:, 0:1].to_broadcast((P, M)))
                    nc.sync.dma_start(
                        out=_sb(hat_d.ap())[:, mt_i, :], in_=sm[:]
                    )
                    fm = pmisc.tile([P, M], F32, tag="fm")
                    nc.vector.tensor_mul(
                        fm[:], sm[:], Ws[:, modal : modal + 1].to_broadcast((P, M))
                    )
                    nc.sync.dma_start(out=_sb(f_d.ap())[:, mt_i, :], in_=fm[:])

    _split_sync_waits(nc)
    return nc


# ---------------------------------------------------------------------------
# host orchestration --------------------------------------------------------
# ---------------------------------------------------------------------------

_cached = {}


def _get_ncs():
    if "nc1" not in _cached:
        _cached["nc1"] = gen_launch1()
        _cached["nc2"] = gen_launch2()
    return _cached["nc1"], _cached["nc2"]


def kernel(
    x, y, w_fc1_t, b_fc1_t, w_fc1_v, b_fc1_v, wq_t, wk_t, wv_t, wq_v, wk_v, wv_v,
    w_fc2_t, b_fc2_t, w_fc2_v, b_fc2_v, w_t, w_v, w, _trace=False,
):
    nc1, nc2 = _get_ncs()

    x = np.asarray(x, dtype=np.float32)
    y = np.asarray(y, dtype=np.float32)

    # gates: unnormalized exp; the 1/(S_t*S_v) scalar rides the exp scale
    et_un = np.exp(np.asarray(w_t, np.float64))
    ev_un = np.exp(np.asarray(w_v, np.float64))
    S_t = float(et_un.sum())
    S_v = float(ev_un.sum())
    c = np.float32(1.0 / (S_t * S_v))

    def bf(a):
        return np.ascontiguousarray(a.astype(BF))

    gt0, gt1 = bf(et_un[0]), bf(et_un[1])
    gv0, gv1 = bf(ev_un[0]), bf(ev_un[1])
    gt2T, gv2T = bf(et_un[2].T), bf(ev_un[2].T)

    shared1 = {
        "w1tT": bf(np.asarray(w_fc1_t, np.float32).T),
        "w1vT": bf(np.asarray(w_fc1_v, np.float32).T),
        "wqtT": bf(np.asarray(wq_t, np.float32).T),
        "wktT": bf(np.asarray(wk_t, np.float32).T),
        "wvtT": bf(np.asarray(wv_t, np.float32).T),
        "wqvT": bf(np.asarray(wq_v, np.float32).T),
        "wkvT": bf(np.asarray(wk_v, np.float32).T),
        "wvvT": bf(np.asarray(wv_v, np.float32).T),
        "gt0": gt0, "gt1": gt1, "gv0": gv0, "gv1": gv1,
        "gt2T": gt2T, "gv2T": gv2T,
        "b1t": np.asarray(b_fc1_t, np.float32).reshape(1, H),
        "b1v": np.asarray(b_fc1_v, np.float32).reshape(1, H),
        "cscal": np.full((P, 1), c, np.float32),
    }

    xT = np.ascontiguousarray(x.transpose(0, 2, 1))  # [B, D, L] f32
    yT = np.ascontiguousarray(y.transpose(0, 2, 1))
    xpT = np.ascontiguousarray(xT.reshape(B, D // 2, 2 * L))  # [b, d2, (dp l)]
    ypT = np.ascontiguousarray(yT.reshape(B, D // 2, 2 * L))

    in_maps1 = []
    for cidx in range(NCORES):
        b0 = BPC * cidx
        m = dict(shared1)
        m["xT"] = bf(xT[b0 : b0 + BPC])
        m["yT"] = bf(yT[b0 : b0 + BPC])
        m["xpT"] = xpT[b0 : b0 + BPC]
        m["ypT"] = ypT[b0 : b0 + BPC]
        in_maps1.append(m)

    r1 = run_bass_kernel_spmd(
        nc1, in_maps1, core_ids=list(range(NCORES)), trace=_trace
    )
    res1 = r1.results

    # ---- host "all-reduce" of the exp partial sums ----
    Sx = float(sum(res1[cc]["sums"][0, 0] for cc in range(NCORES)))
    Sy = float(sum(res1[cc]["sums"][0, 1] for cc in range(NCORES)))
    kx = np.float32(1.0 / (S_v * Sx))
    ky = np.float32(1.0 / (S_t * Sy))
    ew = np.exp(np.asarray(w, np.float64))
    w1 = np.float32(ew[0] / ew.sum())
    w2 = np.float32(ew[1] / ew.sum())

    # reshard: plane p needs batches 2p..2p+3 = cores p, p+1
    ZxT = np.concatenate([res1[cc]["ZxT"] for cc in range(NCORES)], axis=0)
    ZyT = np.concatenate([res1[cc]["ZyT"] for cc in range(NCORES)], axis=0)
    pcat = {
        n: np.concatenate([res1[cc][n] for cc in range(NCORES)], axis=0)
        for n in ("pxn", "pxd", "pyn", "pyd")
    }

    shared2 = {
        "w2vT": np.ascontiguousarray(np.asarray(w_fc2_v, np.float32).T),
        "w2tT": np.ascontiguousarray(np.asarray(w_fc2_t, np.float32).T),
        "b2v": np.asarray(b_fc2_v, np.float32).reshape(1, M),
        "b2t": np.asarray(b_fc2_t, np.float32).reshape(1, M),
        "kscal": np.broadcast_to(
            np.array([[kx, ky]], np.float32), (P, 2)
        ).copy(),
        "wscal": np.broadcast_to(
            np.array([[w1, w2]], np.float32), (P, 2)
        ).copy(),
    }
    in_maps2 = []
    for cidx in range(NCORES):
        p = min(cidx, PLANES - 1)  # core 7 duplicates plane 6; discarded
        m = dict(shared2)
        m["ZxT4"] = ZxT[2 * p : 2 * p + 4]
        m["ZyT4"] = ZyT[2 * p : 2 * p + 4]
        for n in ("pxn", "pxd", "pyn", "pyd"):
            m[n] = pcat[n][2 * p : 2 * p + 4]
        in_maps2.append(m)

    r2 = run_bass_kernel_spmd(
        nc2, in_maps2, core_ids=list(range(NCORES)), trace=_trace
    )
    res2 = r2.results

    x_hat = np.stack([res2[p]["xhat"] for p in range(PLANES)], axis=0)
    y_hat = np.stack([res2[p]["yhat"] for p in range(PLANES)], axis=0)
    fx = np.stack([res2[p]["fxo"] for p in range(PLANES)], axis=0)
    fy = np.stack([res2[p]["fyo"] for p in range(PLANES)], axis=0)
    feature = np.concatenate([fx, fy], axis=-1)

    if _trace:
        kernel._last_exec_ns = (r1.exec_time_ns or 0, r2.exec_time_ns or 0)
    return feature, x_hat, y_hat
